# revision 16
# baseline (speedup 1.0000x reference)
"""Trainium2 Bass kernel for nn_BoxRepelLoss (rotated-box repel/IoU loss).

Same sort-free Green's-theorem IoU as the fp32 baseline (Liang-Barsky slab
clipping of each box's 4 edges against the partner, 8 edge slots per pair),
but the wide per-pair pipeline runs in fp16 to hit the DVE 2x_1p (tensor
ops) / 4x_2p (tensor-scalar ops) fast modes:

  - A phase computes corner projections from PAIR-RELATIVE quantities:
      dca_c = C_clip*(qx_subj +- ddx) + S_clip*(qy_subj +- ddy)
    with qx/qy = host-precomputed corner offsets from the box's own center
    and ddx/ddy = host-precomputed per-pair center deltas. This keeps all
    fp16-rounded magnitudes at box scale (no unit-square cancellation).
  - Edge projections r = adjacent dca differences.  1/r runs as fp32
    reciprocal_approx_fast on the DVE, with up/down casts on the otherwise
    idle Scalar engine (Copy activation; the up-cast adds a 1e-6 bias so r
    is never exactly 0).  The fp16 rinv is clamped to +-3e4 so downstream
    stays finite (near-parallel edges keep exact interval semantics).
  - B/C phases (interval endpoints, clamp, dt) are fp16 tensor_scalar-heavy.
  - dt*K and the area fold stay fp32 (K = cross(corner, edge) has large
    cancellation across the 8 slots).
  - Repel/size/margin epilogues run on the Scalar engine with fused
    accum_out reductions; a PE matmul against ones reduces partitions.

Pair enumeration (identical to baseline): unordered pairs (i, i+k mod m),
k = kt*128 + p + 1 in [1,384], i = 96*d + c; the k=384 row appears twice
and is weighted 0.5 via the per-partition wcol scalar on the kt=2 block.
Host combines per-core partial sums:
  total = 2*S_rep/(m(m-1)) + S_size/m + 2*S_iou/m^2
"""

import numpy as np

M = 768
NDEV = 8
CPD = M // NDEV          # 96 i-columns per core
NKT = 3                  # k-tiles: k = kt*128 + p + 1 in [1, 384]
W288 = NKT * CPD         # 288 pair-columns per partition
W1152 = 4 * W288
W2304 = 2 * W1152
HROW = 224               # hankel window row length per (row, kt)

# fp16 bank row indices
P_QX, P_QY, P_COS, P_SIN, P_W2, P_H2 = 0, 4, 8, 9, 10, 11     # peri16
H_COS, H_SIN, H_QX, H_QY, H_W2, H_H2 = 0, 1, 2, 6, 10, 11     # hank16
NR16 = 12
# fp32 rows: K0-3, a2 (+ wcol row 5 in hank32)
NR32P, NR32H = 5, 6

REPEL_MARGIN = 0.08
MIN_SIZE = 0.02
IOU_MARGIN = 0.1

_PROGRAM_CACHE = {}


def _build_program():
    import concourse.bass as bass
    import concourse.mybir as mybir
    from concourse import bacc
    from concourse.tile import TileContext

    fp32 = mybir.dt.float32
    fp16 = mybir.dt.float16
    Alu = mybir.AluOpType
    Act = mybir.ActivationFunctionType

    nc = bacc.Bacc('TRN2', target_bir_lowering=False, debug=False)
    # hank banks are host-materialized per-partition (contiguous rows per
    # partition -> 128 large DMA descriptors instead of 3072 tiny ones)
    hank16 = nc.dram_tensor('hank16', [128, NR16 * W288], fp16, kind='ExternalInput')
    peri16 = nc.dram_tensor('peri16', [NR16, W288], fp16, kind='ExternalInput')
    hank32 = nc.dram_tensor('hank32', [128, 5 * W288], fp32, kind='ExternalInput')
    peri32 = nc.dram_tensor('peri32', [NR32P, W288], fp32, kind='ExternalInput')
    wcolt = nc.dram_tensor('wcolt', [128, 1], fp32, kind='ExternalInput')
    ddxy = nc.dram_tensor('ddxy', [128, 2 * W288], fp16, kind='ExternalInput')
    out = nc.dram_tensor('out', [6, 1], fp32, kind='ExternalOutput')

    def sub(t, off, free_dims):
        base = t[:]
        return bass.AP(base.tensor, base.offset + off, [list(base.ap[0])] + free_dims)

    with TileContext(nc) as tc:
        with tc.tile_pool(name='p', bufs=1) as pool, \
             tc.tile_pool(name='ps', bufs=1, space='PSUM') as ppool:
            psum6 = ppool.tile([6, 1], fp32, tag='psum6')
            p16 = pool.tile([128, NR16 * W288], fp16, tag='p16')
            h16 = pool.tile([128, NR16 * W288], fp16, tag='h16')
            kk = pool.tile([128, 8 * W288], fp32, tag='kk')
            a2 = pool.tile([128, 2 * W288], fp32, tag='a2')
            wcol = pool.tile([128, 1], fp32, tag='wcol')
            dd = pool.tile([128, 2 * W288], fp16, tag='dd')

            def wt16(tag):
                return pool.tile([128, W2304], fp16, tag=tag, name=tag)

            tu0, tu1 = wt16('tu0'), wt16('tu1')
            dca_c, dca_s = wt16('dca_c'), wt16('dca_s')
            r_c, r_s = wt16('r_c'), wt16('r_s')
            scr = wt16('scr')
            hi_c, hi_s = wt16('hi_c'), wt16('hi_s')
            ri_c, ri_s = wt16('ri_c'), wt16('ri_s')
            ra_c, ra_s = wt16('ra_c'), wt16('ra_s')
            r32_c = pool.tile([128, W2304], fp32, tag='r32_c')
            r32_s = pool.tile([128, W2304], fp32, tag='r32_s')
            q32_c = pool.tile([128, W2304], fp32, tag='q32_c')
            q32_s = pool.tile([128, W2304], fp32, tag='q32_s')
            dtk = pool.tile([128, W2304], fp32, tag='dtk')
            S32 = pool.tile([128, W288], fp32, tag='S32')
            U32 = pool.tile([128, W288], fp32, tag='U32')
            RU = pool.tile([128, W288], fp32, tag='RU')
            scrap = pool.tile([128, W288], fp32, tag='scrap')
            X16 = pool.tile([128, W288], fp16, tag='X16')
            Y16 = pool.tile([128, W288], fp16, tag='Y16')
            z96 = pool.tile([1, CPD], fp16, tag='z96')
            acc = pool.tile([128, 6], fp32, tag='acc')
            red6 = pool.tile([128, 1], fp32, tag='red6')
            ones = pool.tile([128, 1], fp32, tag='ones')

            # const APs for activation biases, seeded on DVE inside the tile
            # context (no all-engine barrier needed; deps are tracked)
            for v in (0.0, REPEL_MARGIN, MIN_SIZE):
                cst = pool.tile([128, 1], fp32, tag=f'cst{v}')
                nc.vector.memset(cst[:], v)
                nc.const_aps.aps[(fp32, v)] = cst[:]

            # ---- DMAs (in consumption order) ----
            def part_in(src, off, n):
                base = src[:]
                return bass.AP(base.tensor, off, [list(base.ap[0]), [1, n]])

            def part_out(dst, off, n):
                base = dst[:]
                return bass.AP(base.tensor, base.offset + off,
                               [list(base.ap[0]), [1, n]])

            def bcast_in(src, r0, n):
                return bass.AP(src[:].tensor, r0 * W288, [[0, 128], [1, n * W288]])

            def bcast_out(dst, r0, n):
                base = dst[:]
                return bass.AP(base.tensor, base.offset + r0 * W288,
                               [list(base.ap[0]), [1, n * W288]])

            # Input DMAs spread across the three DMA-capable queues (SP,
            # Activation HW-DGE, GpSimd SW-DGE) so they run concurrently.
            # sync: the per-partition hank16 bank (cos/sin first)
            nc.sync.dma_start(out=part_out(h16, 0, 2 * W288),
                              in_=part_in(hank16, 0, 2 * W288))
            nc.sync.dma_start(out=part_out(h16, 2 * W288, 10 * W288),
                              in_=part_in(hank16, 2 * W288, 10 * W288))
            # scalar: ddxy + the broadcast peri rows
            nc.scalar.dma_start(out=dd[:], in_=bass.AP(ddxy[:].tensor, 0,
                                [[2 * W288, 128], [1, 2 * W288]]))
            nc.scalar.dma_start(out=bcast_out(p16, P_QX, 8),
                                in_=bcast_in(peri16, P_QX, 8))
            nc.scalar.dma_start(out=bcast_out(p16, P_COS, 4),
                                in_=bcast_in(peri16, P_COS, 4))
            # gpsimd: fp32 K slots / a2 / wcol (needed only by the C phase)
            nc.gpsimd.dma_start(out=bcast_out(kk, 0, 4), in_=bcast_in(peri32, 0, 4))
            nc.gpsimd.dma_start(out=part_out(kk, 4 * W288, 4 * W288),
                                in_=part_in(hank32, 0, 4 * W288))
            nc.gpsimd.dma_start(out=bcast_out(a2, 0, 1), in_=bcast_in(peri32, 4, 1))
            nc.gpsimd.dma_start(out=part_out(a2, W288, W288),
                                in_=part_in(hank32, 4 * W288, W288))
            nc.gpsimd.dma_start(out=wcol[:], in_=part_in(wcolt, 0, 1))

            # ---- view helpers ----
            def crow(bank, r):    # 1 row, e-broadcast as [128, 4, 288]
                return sub(bank, r * W288, [[0, 4], [1, W288]])

            def v4(bank, r0):     # 4 rows as [128, 4, 288]
                return sub(bank, r0 * W288, [[W288, 4], [1, W288]])

            def h4(t, off):       # [4, 288]-shaped half view of a work tile
                return sub(t, off, [[W288, 4], [1, W288]])

            def seg(t, lo, hi):
                return t[:, lo:hi]

            ddx_b = sub(dd, 0, [[0, 4], [1, W288]])
            ddy_b = sub(dd, W288, [[0, 4], [1, W288]])

            tt = nc.vector.tensor_tensor
            ts = nc.vector.tensor_scalar
            act = nc.scalar.activation

            nc.vector.memset(acc[:], 0.0)
            nc.vector.memset(ones[:], 1.0)

            # ---- A phase, direction 0 (subject = i/peri, clip = j/hank) ----
            def a_phase(ho, subj, clip, qx_r, qy_r, cos_r, sin_r, tu, dd_op):
                tt(out=h4(tu, 0), in0=v4(subj, qx_r), in1=ddx_b, op=dd_op)
                tt(out=h4(tu, W1152), in0=v4(subj, qy_r), in1=ddy_b, op=dd_op)
                t_, u_ = h4(tu, 0), h4(tu, W1152)
                tf, uf = seg(tu, 0, W1152), seg(tu, W1152, W2304)
                C, S = crow(clip, cos_r), crow(clip, sin_r)
                tt(out=h4(scr, 0), in0=C, in1=t_, op=Alu.mult)
                tt(out=h4(dca_c, ho), in0=S, in1=u_, op=Alu.mult)
                tt(out=seg(dca_c, ho, ho + W1152), in0=seg(scr, 0, W1152),
                   in1=seg(dca_c, ho, ho + W1152), op=Alu.add)
                tt(out=h4(scr, 0), in0=C, in1=u_, op=Alu.mult)
                tt(out=h4(dca_s, ho), in0=S, in1=t_, op=Alu.mult)
                tt(out=seg(dca_s, ho, ho + W1152), in0=seg(scr, 0, W1152),
                   in1=seg(dca_s, ho, ho + W1152), op=Alu.subtract)
                for dca, rr in ((dca_c, r_c), (dca_s, r_s)):
                    tt(out=seg(rr, ho, ho + 3 * W288),
                       in0=seg(dca, ho + W288, ho + W1152),
                       in1=seg(dca, ho, ho + 3 * W288), op=Alu.subtract)
                    tt(out=seg(rr, ho + 3 * W288, ho + W1152),
                       in0=seg(dca, ho, ho + W288),
                       in1=seg(dca, ho + 3 * W288, ho + W1152), op=Alu.subtract)

            a_phase(0, p16, h16, P_QX, P_QY, H_COS, H_SIN, tu0, Alu.add)
            # r -> fp32 (+1e-6 so never exactly 0) on Scalar; recip on DVE
            act(out=seg(r32_c, 0, W1152), in_=seg(r_c, 0, W1152),
                func=Act.Copy, bias=1e-6)
            act(out=seg(r32_s, 0, W1152), in_=seg(r_s, 0, W1152),
                func=Act.Copy, bias=1e-6)
            # repel dist^2 (independent; fills the recip latency)
            tt(out=X16[:], in0=seg(dd, 0, W288), in1=seg(dd, 0, W288),
               op=Alu.mult)
            tt(out=Y16[:], in0=seg(dd, W288, 2 * W288),
               in1=seg(dd, W288, 2 * W288), op=Alu.mult)
            tt(out=X16[:], in0=X16[:], in1=Y16[:], op=Alu.add)
            nc.vector.reciprocal_approx_fast(out=seg(q32_c, 0, W1152),
                                             in_=seg(r32_c, 0, W1152))
            nc.vector.reciprocal_approx_fast(out=seg(q32_s, 0, W1152),
                                             in_=seg(r32_s, 0, W1152))
            act(out=seg(ri_c, 0, W1152), in_=seg(q32_c, 0, W1152), func=Act.Copy)
            act(out=seg(ra_c, 0, W1152), in_=seg(q32_c, 0, W1152), func=Act.Abs)
            act(out=seg(ri_s, 0, W1152), in_=seg(q32_s, 0, W1152), func=Act.Copy)
            act(out=seg(ra_s, 0, W1152), in_=seg(q32_s, 0, W1152), func=Act.Abs)

            a_phase(W1152, h16, p16, H_QX, H_QY, P_COS, P_SIN, tu1, Alu.subtract)
            act(out=seg(r32_c, W1152, W2304), in_=seg(r_c, W1152, W2304),
                func=Act.Copy, bias=1e-6)
            act(out=seg(r32_s, W1152, W2304), in_=seg(r_s, W1152, W2304),
                func=Act.Copy, bias=1e-6)
            nc.vector.reciprocal_approx_fast(out=seg(q32_c, W1152, W2304),
                                             in_=seg(r32_c, W1152, W2304))
            nc.vector.reciprocal_approx_fast(out=seg(q32_s, W1152, W2304),
                                             in_=seg(r32_s, W1152, W2304))
            act(out=seg(ri_c, W1152, W2304), in_=seg(q32_c, W1152, W2304),
                func=Act.Copy)
            act(out=seg(ra_c, W1152, W2304), in_=seg(q32_c, W1152, W2304),
                func=Act.Abs)
            act(out=seg(ri_s, W1152, W2304), in_=seg(q32_s, W1152, W2304),
                func=Act.Copy)
            act(out=seg(ra_s, W1152, W2304), in_=seg(q32_s, W1152, W2304),
                func=Act.Abs)

            # ---- B phase per axis: g, habs, hi, Aa (lo = -Aa) ----
            # signed rinv and |rinv| both clamped at the same 3e4 scale so
            # |g| vs habs comparisons keep exact interval semantics even for
            # near-parallel edges (huge rinv).
            for dca, rr, ri, ra, hi_t, w2r_h, w2r_p in (
                    (dca_c, r_c, ri_c, ra_c, hi_c, H_W2, P_W2),
                    (dca_s, r_s, ri_s, ra_s, hi_s, H_H2, P_H2)):
                ts(out=ri[:], in0=ri[:], scalar1=-30000.0, scalar2=30000.0,
                   op0=Alu.max, op1=Alu.min)
                ts(out=ra[:], in0=ra[:], scalar1=30000.0, scalar2=None,
                   op0=Alu.min)
                tt(out=dca[:], in0=dca[:], in1=ri[:], op=Alu.mult)       # g
                tt(out=h4(rr, 0), in0=crow(h16, w2r_h), in1=h4(ra, 0),
                   op=Alu.mult)                                           # habs
                tt(out=h4(rr, W1152), in0=crow(p16, w2r_p),
                   in1=h4(ra, W1152), op=Alu.mult)
                tt(out=hi_t[:], in0=rr[:], in1=dca[:], op=Alu.subtract)   # hi
                tt(out=dca[:], in0=rr[:], in1=dca[:], op=Alu.add)         # Aa

            # ---- C phase: dt then dt*K (fp32) ----
            tt(out=hi_c[:], in0=hi_c[:], in1=hi_s[:], op=Alu.min)         # HI
            ts(out=hi_c[:], in0=hi_c[:], scalar1=1.0, scalar2=0.0,
               op0=Alu.min, op1=Alu.max)                                  # HI'
            tt(out=dca_c[:], in0=dca_c[:], in1=dca_s[:], op=Alu.min)      # LOn
            ts(out=dca_c[:], in0=dca_c[:], scalar1=-1.0, scalar2=0.0,
               op0=Alu.mult, op1=Alu.max)                                 # LO'
            tt(out=hi_c[:], in0=hi_c[:], in1=dca_c[:], op=Alu.subtract)
            ts(out=hi_c[:], in0=hi_c[:], scalar1=0.0, scalar2=1.0,
               op0=Alu.max, op1=Alu.min)                                  # dt
            tt(out=dtk[:], in0=hi_c[:], in1=kk[:], op=Alu.mult)
            tt(out=seg(dtk, 0, W1152), in0=seg(dtk, 0, W1152),
               in1=seg(dtk, W1152, W2304), op=Alu.add)
            tt(out=seg(dtk, 0, 2 * W288), in0=seg(dtk, 0, 2 * W288),
               in1=seg(dtk, 2 * W288, W1152), op=Alu.add)
            tt(out=S32[:], in0=seg(dtk, 0, W288), in1=seg(dtk, W288, 2 * W288),
               op=Alu.add)                                                # 2*inter

            # ---- IoU epilogue ----
            tt(out=U32[:], in0=seg(a2, 0, W288), in1=seg(a2, W288, 2 * W288),
               op=Alu.add)
            tt(out=U32[:], in0=U32[:], in1=S32[:], op=Alu.subtract)       # union2
            nc.vector.reciprocal_approx_fast(out=RU[:], in_=U32[:])
            tt(out=RU[:], in0=S32[:], in1=RU[:], op=Alu.mult)             # iou
            ts(out=RU[:], in0=RU[:], scalar1=IOU_MARGIN, scalar2=0.0,
               op0=Alu.subtract, op1=Alu.max)                 # relu(iou-0.1)
            ts(out=seg(scrap, 0, 2 * CPD), in0=seg(RU, 0, 2 * CPD),
               scalar1=1.0, scalar2=0.0, op0=Alu.mult,
               op1=Alu.add, accum_out=acc[:, 0:1])
            ts(out=seg(scrap, 0, CPD), in0=seg(RU, 2 * CPD, W288),
               scalar1=wcol[:, 0:1], scalar2=0.0, op0=Alu.mult,
               op1=Alu.add, accum_out=acc[:, 1:2])

            # ---- repel epilogue (dist^2 computed above) ----
            act(out=X16[:], in_=X16[:], func=Act.Sqrt)
            act(out=X16[:], in_=X16[:], func=Act.Relu,
                bias=REPEL_MARGIN, scale=-1.0)
            ts(out=seg(Y16, 0, 2 * CPD), in0=seg(X16, 0, 2 * CPD),
               scalar1=1.0, scalar2=0.0, op0=Alu.mult, op1=Alu.add,
               accum_out=acc[:, 2:3])
            ts(out=seg(Y16, 0, CPD), in0=seg(X16, 2 * CPD, W288),
               scalar1=wcol[:, 0:1], scalar2=0.0, op0=Alu.mult, op1=Alu.add,
               accum_out=acc[:, 3:4])

            # ---- size penalty (this core's 96 boxes; partition 0) ----
            act(out=z96[:], in_=p16[0:1, P_W2 * W288:P_W2 * W288 + CPD],
                func=Act.Relu, bias=MIN_SIZE, scale=-2.0,
                accum_out=acc[0:1, 4:5])
            act(out=z96[:], in_=p16[0:1, P_H2 * W288:P_H2 * W288 + CPD],
                func=Act.Relu, bias=MIN_SIZE, scale=-2.0,
                accum_out=acc[0:1, 5:6])

            # ---- partition reduction via PE, then DMA out ----
            nc.tensor.matmul(out=psum6[:], lhsT=acc[:], rhs=ones[:],
                             start=True, stop=True)
            act(out=red6[0:6, 0:1], in_=psum6[:], func=Act.Copy)
            nc.sync.dma_start(out=out[:], in_=red6[0:6, 0:1])
    nc.compile()
    return nc


def _features(pred):
    """Host feature tables: (F16 [12, M], F32 [5, M]) fp32 values."""
    p = np.asarray(pred, np.float32)[:-1]
    cx, cy, w, h = p[:, 0], p[:, 1], p[:, 2], p[:, 3]
    th = np.arctan2(p[:, 5], p[:, 4]).astype(np.float32)
    c = np.cos(th).astype(np.float32)
    s = np.sin(th).astype(np.float32)
    dx = np.stack([-w, w, w, -w], 0) * np.float32(0.5)   # [4, M]
    dy = np.stack([-h, -h, h, h], 0) * np.float32(0.5)
    qx = c[None] * dx - s[None] * dy                      # corner - center
    qy = s[None] * dx + c[None] * dy
    xa = cx[None] + qx
    ya = cy[None] + qy
    ex = np.roll(xa, -1, 0) - xa
    ey = np.roll(ya, -1, 0) - ya
    K = xa * ey - ya * ex
    F16 = np.empty((NR16, M), np.float32)
    F16[P_QX:P_QX + 4] = qx
    F16[P_QY:P_QY + 4] = qy
    F16[P_COS], F16[P_SIN] = c, s
    F16[P_W2], F16[P_H2] = w * 0.5, h * 0.5
    F32 = np.empty((5, M), np.float32)
    F32[0:4] = K
    F32[4] = 2.0 * w * h
    return F16, F32, cx, cy


# hank16 row order: cos, sin, qx0-3, qy0-3, w2, h2 (indices into F16 rows)
_H16_SRC = [P_COS, P_SIN] + list(range(P_QX, P_QX + 4)) + \
           list(range(P_QY, P_QY + 4)) + [P_W2, P_H2]


def _prep_inputs(pred):
    from numpy.lib.stride_tricks import sliding_window_view
    F16, F32, cx, cy = _features(pred)
    # window max index: d*96 + kt*128 + p + 1 + c <= 671 + 256 + 127 + 1 + 95
    Fe16 = np.concatenate([F16, F16[:, :M // 2]], 1)
    Fe32 = np.concatenate([F32, F32[:, :M // 2]], 1)
    wcol = np.ones((128, 1), np.float32)
    wcol[127, 0] = 0.5          # partition 127 kt=2 holds the k=384 dup
    p_ = np.arange(128)[:, None, None]
    kt_ = np.arange(NKT)[None, :, None]
    c_ = np.arange(CPD)[None, None, :]
    in_maps = []
    for d in range(NDEV):
        h16 = np.empty((128, NR16 * W288), np.float16)
        h32 = np.empty((128, 5 * W288), np.float32)
        for ri, r in enumerate(_H16_SRC):
            for kt in range(NKT):
                lo = d * CPD + kt * 128 + 1
                sw = sliding_window_view(Fe16[r, lo:lo + 127 + CPD], CPD)
                h16[:, ri * W288 + kt * CPD:ri * W288 + (kt + 1) * CPD] = sw
        for r in range(5):
            for kt in range(NKT):
                lo = d * CPD + kt * 128 + 1
                sw = sliding_window_view(Fe32[r, lo:lo + 127 + CPD], CPD)
                h32[:, r * W288 + kt * CPD:r * W288 + (kt + 1) * CPD] = sw
        p16 = np.tile(F16[:, d * CPD:(d + 1) * CPD], (1, NKT)).astype(np.float16)
        p32 = np.tile(F32[:, d * CPD:(d + 1) * CPD], (1, NKT))
        i_ = d * CPD + c_
        j_ = (i_ + kt_ * 128 + p_ + 1) % M
        dd = np.empty((128, 2 * W288), np.float16)
        dd[:, 0:W288] = (cx[i_] - cx[j_]).reshape(128, W288)
        dd[:, W288:] = (cy[i_] - cy[j_]).reshape(128, W288)
        in_maps.append({
            'hank16': h16, 'peri16': np.ascontiguousarray(p16),
            'hank32': h32, 'peri32': np.ascontiguousarray(p32),
            'wcolt': wcol, 'ddxy': dd,
        })
    return in_maps


def _combine(partials):
    m = float(M)
    S_iou = sum(float(p[0, 0]) + float(p[1, 0]) for p in partials)
    S_rep = sum(float(p[2, 0]) + float(p[3, 0]) for p in partials)
    S_size = sum(float(p[4, 0]) + float(p[5, 0]) for p in partials)
    return np.array((2.0 * S_rep) / (m * (m - 1.0)) + S_size / m
                    + (2.0 * S_iou) / (m * m), dtype=np.float32)


def kernel(pred):
    from concourse import bass_utils
    if 'nc' not in _PROGRAM_CACHE:
        _PROGRAM_CACHE['nc'] = _build_program()
    nc = _PROGRAM_CACHE['nc']
    in_maps = _prep_inputs(pred)
    res = bass_utils.run_bass_kernel_spmd(nc, in_maps, core_ids=list(range(NDEV)))
    return _combine([r['out'] for r in res.results])


if __name__ == '__main__':
    pred = np.load('/root/problem/pred.npy')
    print('kernel total:', kernel(pred))


# revision 17
# speedup vs baseline: 1.0263x; 1.0263x over previous
"""Trainium2 Bass kernel for nn_BoxRepelLoss (rotated-box repel/IoU loss).

Same sort-free Green's-theorem IoU as the fp32 baseline (Liang-Barsky slab
clipping of each box's 4 edges against the partner, 8 edge slots per pair),
but the wide per-pair pipeline runs in fp16 to hit the DVE 2x_1p (tensor
ops) / 4x_2p (tensor-scalar ops) fast modes:

  - A phase computes corner projections from PAIR-RELATIVE quantities:
      dca_c = C_clip*(qx_subj +- ddx) + S_clip*(qy_subj +- ddy)
    with qx/qy = host-precomputed corner offsets from the box's own center
    and ddx/ddy = host-precomputed per-pair center deltas. This keeps all
    fp16-rounded magnitudes at box scale (no unit-square cancellation).
  - Edge projections r = adjacent dca differences.  1/r runs as fp32
    reciprocal_approx_fast on the DVE, with up/down casts on the otherwise
    idle Scalar engine (Copy activation; the up-cast adds a 1e-6 bias so r
    is never exactly 0).  The fp16 rinv is clamped to +-3e4 so downstream
    stays finite (near-parallel edges keep exact interval semantics).
  - B/C phases (interval endpoints, clamp, dt) are fp16 tensor_scalar-heavy.
  - dt*K and the area fold stay fp32 (K = cross(corner, edge) has large
    cancellation across the 8 slots).
  - Repel/size/margin epilogues run on the Scalar engine with fused
    accum_out reductions; a PE matmul against ones reduces partitions.

Pair enumeration (identical to baseline): unordered pairs (i, i+k mod m),
k = kt*128 + p + 1 in [1,384], i = 96*d + c; the k=384 row appears twice
and is weighted 0.5 via the per-partition wcol scalar on the kt=2 block.
Host combines per-core partial sums:
  total = 2*S_rep/(m(m-1)) + S_size/m + 2*S_iou/m^2
"""

import numpy as np

M = 768
NDEV = 8
CPD = M // NDEV          # 96 i-columns per core
NKT = 3                  # k-tiles: k = kt*128 + p + 1 in [1, 384]
W288 = NKT * CPD         # 288 pair-columns per partition
W1152 = 4 * W288
W2304 = 2 * W1152
HROW = 224               # hankel window row length per (row, kt)

# fp16 bank row indices
P_QX, P_QY, P_COS, P_SIN, P_W2, P_H2 = 0, 4, 8, 9, 10, 11     # peri16
H_COS, H_SIN, H_QX, H_QY, H_W2, H_H2 = 0, 1, 2, 6, 10, 11     # hank16
NR16 = 12
# fp32 rows: K0-3, a2 (+ wcol row 5 in hank32)
NR32P, NR32H = 5, 6

REPEL_MARGIN = 0.08
MIN_SIZE = 0.02
IOU_MARGIN = 0.1

_PROGRAM_CACHE = {}


def _build_program():
    import concourse.bass as bass
    import concourse.mybir as mybir
    from concourse import bacc
    from concourse.tile import TileContext

    fp32 = mybir.dt.float32
    fp16 = mybir.dt.float16
    Alu = mybir.AluOpType
    Act = mybir.ActivationFunctionType

    nc = bacc.Bacc('TRN2', target_bir_lowering=False, debug=False)
    # hank banks are host-materialized per-partition (contiguous rows per
    # partition -> 128 large DMA descriptors instead of 3072 tiny ones)
    hank16 = nc.dram_tensor('hank16', [128, NR16 * W288], fp16, kind='ExternalInput')
    peri16 = nc.dram_tensor('peri16', [NR16, W288], fp16, kind='ExternalInput')
    hank32 = nc.dram_tensor('hank32', [128, 5 * W288], fp32, kind='ExternalInput')
    peri32 = nc.dram_tensor('peri32', [NR32P, W288], fp32, kind='ExternalInput')
    wcolt = nc.dram_tensor('wcolt', [128, 1], fp32, kind='ExternalInput')
    ddxy = nc.dram_tensor('ddxy', [128, 2 * W288], fp16, kind='ExternalInput')
    out = nc.dram_tensor('out', [6, 1], fp32, kind='ExternalOutput')

    def sub(t, off, free_dims):
        base = t[:]
        return bass.AP(base.tensor, base.offset + off, [list(base.ap[0])] + free_dims)

    with TileContext(nc) as tc:
        with tc.tile_pool(name='p', bufs=1) as pool, \
             tc.tile_pool(name='ps', bufs=1, space='PSUM') as ppool:
            psum6 = ppool.tile([6, 1], fp32, tag='psum6')
            p16 = pool.tile([128, NR16 * W288], fp16, tag='p16')
            h16 = pool.tile([128, NR16 * W288], fp16, tag='h16')
            kk = pool.tile([128, 8 * W288], fp32, tag='kk')
            a2 = pool.tile([128, 2 * W288], fp32, tag='a2')
            wcol = pool.tile([128, 1], fp32, tag='wcol')
            dd = pool.tile([128, 2 * W288], fp16, tag='dd')

            def wt16(tag):
                return pool.tile([128, W2304], fp16, tag=tag, name=tag)

            tu0, tu1 = wt16('tu0'), wt16('tu1')
            dca_c, dca_s = wt16('dca_c'), wt16('dca_s')
            r_c, r_s = wt16('r_c'), wt16('r_s')
            scr = wt16('scr')
            hi_c, hi_s = wt16('hi_c'), wt16('hi_s')
            ri_c, ri_s = wt16('ri_c'), wt16('ri_s')
            ra_c, ra_s = wt16('ra_c'), wt16('ra_s')
            r32_c = pool.tile([128, W2304], fp32, tag='r32_c')
            r32_s = pool.tile([128, W2304], fp32, tag='r32_s')
            q32_c = pool.tile([128, W2304], fp32, tag='q32_c')
            q32_s = pool.tile([128, W2304], fp32, tag='q32_s')
            dtk = pool.tile([128, W2304], fp32, tag='dtk')
            S32 = pool.tile([128, W288], fp32, tag='S32')
            U32 = pool.tile([128, W288], fp32, tag='U32')
            RU = pool.tile([128, W288], fp32, tag='RU')
            scrap = pool.tile([128, W288], fp32, tag='scrap')
            X16 = pool.tile([128, W288], fp16, tag='X16')
            Y16 = pool.tile([128, W288], fp16, tag='Y16')
            z96 = pool.tile([1, CPD], fp16, tag='z96')
            acc = pool.tile([128, 6], fp32, tag='acc')
            red6 = pool.tile([128, 1], fp32, tag='red6')
            ones = pool.tile([128, 1], fp32, tag='ones')

            # const APs for activation biases, seeded on DVE inside the tile
            # context (no all-engine barrier needed; deps are tracked)
            for v in (0.0, REPEL_MARGIN, MIN_SIZE):
                cst = pool.tile([128, 1], fp32, tag=f'cst{v}')
                nc.vector.memset(cst[:], v)
                nc.const_aps.aps[(fp32, v)] = cst[:]

            # ---- DMAs (in consumption order) ----
            def part_in(src, off, n):
                base = src[:]
                return bass.AP(base.tensor, off, [list(base.ap[0]), [1, n]])

            def part_out(dst, off, n):
                base = dst[:]
                return bass.AP(base.tensor, base.offset + off,
                               [list(base.ap[0]), [1, n]])

            def bcast_in(src, r0, n):
                return bass.AP(src[:].tensor, r0 * W288, [[0, 128], [1, n * W288]])

            def bcast_out(dst, r0, n):
                base = dst[:]
                return bass.AP(base.tensor, base.offset + r0 * W288,
                               [list(base.ap[0]), [1, n * W288]])

            # Input DMAs spread across the two HW-DGE queues (SP + Activation)
            # so they run concurrently.  (GpSimd SW-DGE has ~10us completion
            # latency -- measured -- so it is not used.)
            # sync: the per-partition hank16 bank (cos/sin first) + fp32 banks
            nc.sync.dma_start(out=part_out(h16, 0, 2 * W288),
                              in_=part_in(hank16, 0, 2 * W288))
            nc.sync.dma_start(out=part_out(h16, 2 * W288, 10 * W288),
                              in_=part_in(hank16, 2 * W288, 10 * W288))
            nc.sync.dma_start(out=part_out(kk, 4 * W288, 4 * W288),
                              in_=part_in(hank32, 0, 4 * W288))
            nc.sync.dma_start(out=part_out(a2, W288, W288),
                              in_=part_in(hank32, 4 * W288, W288))
            nc.sync.dma_start(out=wcol[:], in_=part_in(wcolt, 0, 1))
            # scalar: ddxy + the broadcast peri rows + fp32 peri slots
            nc.scalar.dma_start(out=dd[:], in_=bass.AP(ddxy[:].tensor, 0,
                                [[2 * W288, 128], [1, 2 * W288]]))
            nc.scalar.dma_start(out=bcast_out(p16, P_QX, 8),
                                in_=bcast_in(peri16, P_QX, 8))
            nc.scalar.dma_start(out=bcast_out(p16, P_COS, 4),
                                in_=bcast_in(peri16, P_COS, 4))
            nc.scalar.dma_start(out=bcast_out(kk, 0, 4), in_=bcast_in(peri32, 0, 4))
            nc.scalar.dma_start(out=bcast_out(a2, 0, 1), in_=bcast_in(peri32, 4, 1))

            # ---- view helpers ----
            def crow(bank, r):    # 1 row, e-broadcast as [128, 4, 288]
                return sub(bank, r * W288, [[0, 4], [1, W288]])

            def v4(bank, r0):     # 4 rows as [128, 4, 288]
                return sub(bank, r0 * W288, [[W288, 4], [1, W288]])

            def h4(t, off):       # [4, 288]-shaped half view of a work tile
                return sub(t, off, [[W288, 4], [1, W288]])

            def seg(t, lo, hi):
                return t[:, lo:hi]

            ddx_b = sub(dd, 0, [[0, 4], [1, W288]])
            ddy_b = sub(dd, W288, [[0, 4], [1, W288]])

            tt = nc.vector.tensor_tensor
            ts = nc.vector.tensor_scalar
            act = nc.scalar.activation

            nc.vector.memset(acc[:], 0.0)
            nc.vector.memset(ones[:], 1.0)

            # ---- A phase, direction 0 (subject = i/peri, clip = j/hank) ----
            def a_phase(ho, subj, clip, qx_r, qy_r, cos_r, sin_r, tu, dd_op):
                tt(out=h4(tu, 0), in0=v4(subj, qx_r), in1=ddx_b, op=dd_op)
                tt(out=h4(tu, W1152), in0=v4(subj, qy_r), in1=ddy_b, op=dd_op)
                t_, u_ = h4(tu, 0), h4(tu, W1152)
                tf, uf = seg(tu, 0, W1152), seg(tu, W1152, W2304)
                C, S = crow(clip, cos_r), crow(clip, sin_r)
                tt(out=h4(scr, 0), in0=C, in1=t_, op=Alu.mult)
                tt(out=h4(dca_c, ho), in0=S, in1=u_, op=Alu.mult)
                tt(out=seg(dca_c, ho, ho + W1152), in0=seg(scr, 0, W1152),
                   in1=seg(dca_c, ho, ho + W1152), op=Alu.add)
                tt(out=h4(scr, 0), in0=C, in1=u_, op=Alu.mult)
                tt(out=h4(dca_s, ho), in0=S, in1=t_, op=Alu.mult)
                tt(out=seg(dca_s, ho, ho + W1152), in0=seg(scr, 0, W1152),
                   in1=seg(dca_s, ho, ho + W1152), op=Alu.subtract)
                for dca, rr in ((dca_c, r_c), (dca_s, r_s)):
                    tt(out=seg(rr, ho, ho + 3 * W288),
                       in0=seg(dca, ho + W288, ho + W1152),
                       in1=seg(dca, ho, ho + 3 * W288), op=Alu.subtract)
                    tt(out=seg(rr, ho + 3 * W288, ho + W1152),
                       in0=seg(dca, ho, ho + W288),
                       in1=seg(dca, ho + 3 * W288, ho + W1152), op=Alu.subtract)

            a_phase(0, p16, h16, P_QX, P_QY, H_COS, H_SIN, tu0, Alu.add)
            # r -> fp32 (+1e-6 so never exactly 0) on Scalar; recip on DVE
            act(out=seg(r32_c, 0, W1152), in_=seg(r_c, 0, W1152),
                func=Act.Copy, bias=1e-6)
            act(out=seg(r32_s, 0, W1152), in_=seg(r_s, 0, W1152),
                func=Act.Copy, bias=1e-6)
            # repel dist^2 (independent; fills the recip latency)
            tt(out=X16[:], in0=seg(dd, 0, W288), in1=seg(dd, 0, W288),
               op=Alu.mult)
            tt(out=Y16[:], in0=seg(dd, W288, 2 * W288),
               in1=seg(dd, W288, 2 * W288), op=Alu.mult)
            tt(out=X16[:], in0=X16[:], in1=Y16[:], op=Alu.add)
            nc.vector.reciprocal_approx_fast(out=seg(q32_c, 0, W1152),
                                             in_=seg(r32_c, 0, W1152))
            nc.vector.reciprocal_approx_fast(out=seg(q32_s, 0, W1152),
                                             in_=seg(r32_s, 0, W1152))
            act(out=seg(ri_c, 0, W1152), in_=seg(q32_c, 0, W1152), func=Act.Copy)
            act(out=seg(ra_c, 0, W1152), in_=seg(q32_c, 0, W1152), func=Act.Abs)
            act(out=seg(ri_s, 0, W1152), in_=seg(q32_s, 0, W1152), func=Act.Copy)
            act(out=seg(ra_s, 0, W1152), in_=seg(q32_s, 0, W1152), func=Act.Abs)

            a_phase(W1152, h16, p16, H_QX, H_QY, P_COS, P_SIN, tu1, Alu.subtract)
            act(out=seg(r32_c, W1152, W2304), in_=seg(r_c, W1152, W2304),
                func=Act.Copy, bias=1e-6)
            act(out=seg(r32_s, W1152, W2304), in_=seg(r_s, W1152, W2304),
                func=Act.Copy, bias=1e-6)
            nc.vector.reciprocal_approx_fast(out=seg(q32_c, W1152, W2304),
                                             in_=seg(r32_c, W1152, W2304))
            nc.vector.reciprocal_approx_fast(out=seg(q32_s, W1152, W2304),
                                             in_=seg(r32_s, W1152, W2304))
            act(out=seg(ri_c, W1152, W2304), in_=seg(q32_c, W1152, W2304),
                func=Act.Copy)
            act(out=seg(ra_c, W1152, W2304), in_=seg(q32_c, W1152, W2304),
                func=Act.Abs)
            act(out=seg(ri_s, W1152, W2304), in_=seg(q32_s, W1152, W2304),
                func=Act.Copy)
            act(out=seg(ra_s, W1152, W2304), in_=seg(q32_s, W1152, W2304),
                func=Act.Abs)

            # ---- B phase per axis: g, habs, hi, Aa (lo = -Aa) ----
            # signed rinv and |rinv| both clamped at the same 3e4 scale so
            # |g| vs habs comparisons keep exact interval semantics even for
            # near-parallel edges (huge rinv).
            for dca, rr, ri, ra, hi_t, w2r_h, w2r_p in (
                    (dca_c, r_c, ri_c, ra_c, hi_c, H_W2, P_W2),
                    (dca_s, r_s, ri_s, ra_s, hi_s, H_H2, P_H2)):
                ts(out=ri[:], in0=ri[:], scalar1=-30000.0, scalar2=30000.0,
                   op0=Alu.max, op1=Alu.min)
                ts(out=ra[:], in0=ra[:], scalar1=30000.0, scalar2=None,
                   op0=Alu.min)
                tt(out=dca[:], in0=dca[:], in1=ri[:], op=Alu.mult)       # g
                tt(out=h4(rr, 0), in0=crow(h16, w2r_h), in1=h4(ra, 0),
                   op=Alu.mult)                                           # habs
                tt(out=h4(rr, W1152), in0=crow(p16, w2r_p),
                   in1=h4(ra, W1152), op=Alu.mult)
                tt(out=hi_t[:], in0=rr[:], in1=dca[:], op=Alu.subtract)   # hi
                tt(out=dca[:], in0=rr[:], in1=dca[:], op=Alu.add)         # Aa

            # ---- C phase: dt then dt*K (fp32) ----
            tt(out=hi_c[:], in0=hi_c[:], in1=hi_s[:], op=Alu.min)         # HI
            ts(out=hi_c[:], in0=hi_c[:], scalar1=1.0, scalar2=0.0,
               op0=Alu.min, op1=Alu.max)                                  # HI'
            tt(out=dca_c[:], in0=dca_c[:], in1=dca_s[:], op=Alu.min)      # LOn
            ts(out=dca_c[:], in0=dca_c[:], scalar1=-1.0, scalar2=0.0,
               op0=Alu.mult, op1=Alu.max)                                 # LO'
            tt(out=hi_c[:], in0=hi_c[:], in1=dca_c[:], op=Alu.subtract)
            ts(out=hi_c[:], in0=hi_c[:], scalar1=0.0, scalar2=1.0,
               op0=Alu.max, op1=Alu.min)                                  # dt
            tt(out=dtk[:], in0=hi_c[:], in1=kk[:], op=Alu.mult)
            tt(out=seg(dtk, 0, W1152), in0=seg(dtk, 0, W1152),
               in1=seg(dtk, W1152, W2304), op=Alu.add)
            tt(out=seg(dtk, 0, 2 * W288), in0=seg(dtk, 0, 2 * W288),
               in1=seg(dtk, 2 * W288, W1152), op=Alu.add)
            tt(out=S32[:], in0=seg(dtk, 0, W288), in1=seg(dtk, W288, 2 * W288),
               op=Alu.add)                                                # 2*inter

            # ---- IoU epilogue ----
            tt(out=U32[:], in0=seg(a2, 0, W288), in1=seg(a2, W288, 2 * W288),
               op=Alu.add)
            tt(out=U32[:], in0=U32[:], in1=S32[:], op=Alu.subtract)       # union2
            nc.vector.reciprocal_approx_fast(out=RU[:], in_=U32[:])
            tt(out=RU[:], in0=S32[:], in1=RU[:], op=Alu.mult)             # iou
            ts(out=RU[:], in0=RU[:], scalar1=IOU_MARGIN, scalar2=0.0,
               op0=Alu.subtract, op1=Alu.max)                 # relu(iou-0.1)
            ts(out=seg(scrap, 0, 2 * CPD), in0=seg(RU, 0, 2 * CPD),
               scalar1=1.0, scalar2=0.0, op0=Alu.mult,
               op1=Alu.add, accum_out=acc[:, 0:1])
            ts(out=seg(scrap, 0, CPD), in0=seg(RU, 2 * CPD, W288),
               scalar1=wcol[:, 0:1], scalar2=0.0, op0=Alu.mult,
               op1=Alu.add, accum_out=acc[:, 1:2])

            # ---- repel epilogue (dist^2 computed above) ----
            act(out=X16[:], in_=X16[:], func=Act.Sqrt)
            act(out=X16[:], in_=X16[:], func=Act.Relu,
                bias=REPEL_MARGIN, scale=-1.0)
            ts(out=seg(Y16, 0, 2 * CPD), in0=seg(X16, 0, 2 * CPD),
               scalar1=1.0, scalar2=0.0, op0=Alu.mult, op1=Alu.add,
               accum_out=acc[:, 2:3])
            ts(out=seg(Y16, 0, CPD), in0=seg(X16, 2 * CPD, W288),
               scalar1=wcol[:, 0:1], scalar2=0.0, op0=Alu.mult, op1=Alu.add,
               accum_out=acc[:, 3:4])

            # ---- size penalty (this core's 96 boxes; partition 0) ----
            act(out=z96[:], in_=p16[0:1, P_W2 * W288:P_W2 * W288 + CPD],
                func=Act.Relu, bias=MIN_SIZE, scale=-2.0,
                accum_out=acc[0:1, 4:5])
            act(out=z96[:], in_=p16[0:1, P_H2 * W288:P_H2 * W288 + CPD],
                func=Act.Relu, bias=MIN_SIZE, scale=-2.0,
                accum_out=acc[0:1, 5:6])

            # ---- partition reduction via PE, then DMA out ----
            nc.tensor.matmul(out=psum6[:], lhsT=acc[:], rhs=ones[:],
                             start=True, stop=True)
            act(out=red6[0:6, 0:1], in_=psum6[:], func=Act.Copy)
            nc.sync.dma_start(out=out[:], in_=red6[0:6, 0:1])
    nc.compile()
    return nc


def _features(pred):
    """Host feature tables: (F16 [12, M], F32 [5, M]) fp32 values."""
    p = np.asarray(pred, np.float32)[:-1]
    cx, cy, w, h = p[:, 0], p[:, 1], p[:, 2], p[:, 3]
    th = np.arctan2(p[:, 5], p[:, 4]).astype(np.float32)
    c = np.cos(th).astype(np.float32)
    s = np.sin(th).astype(np.float32)
    dx = np.stack([-w, w, w, -w], 0) * np.float32(0.5)   # [4, M]
    dy = np.stack([-h, -h, h, h], 0) * np.float32(0.5)
    qx = c[None] * dx - s[None] * dy                      # corner - center
    qy = s[None] * dx + c[None] * dy
    xa = cx[None] + qx
    ya = cy[None] + qy
    ex = np.roll(xa, -1, 0) - xa
    ey = np.roll(ya, -1, 0) - ya
    K = xa * ey - ya * ex
    F16 = np.empty((NR16, M), np.float32)
    F16[P_QX:P_QX + 4] = qx
    F16[P_QY:P_QY + 4] = qy
    F16[P_COS], F16[P_SIN] = c, s
    F16[P_W2], F16[P_H2] = w * 0.5, h * 0.5
    F32 = np.empty((5, M), np.float32)
    F32[0:4] = K
    F32[4] = 2.0 * w * h
    return F16, F32, cx, cy


# hank16 row order: cos, sin, qx0-3, qy0-3, w2, h2 (indices into F16 rows)
_H16_SRC = [P_COS, P_SIN] + list(range(P_QX, P_QX + 4)) + \
           list(range(P_QY, P_QY + 4)) + [P_W2, P_H2]


def _prep_inputs(pred):
    from numpy.lib.stride_tricks import sliding_window_view
    F16, F32, cx, cy = _features(pred)
    # window max index: d*96 + kt*128 + p + 1 + c <= 671 + 256 + 127 + 1 + 95
    Fe16 = np.concatenate([F16, F16[:, :M // 2]], 1)
    Fe32 = np.concatenate([F32, F32[:, :M // 2]], 1)
    wcol = np.ones((128, 1), np.float32)
    wcol[127, 0] = 0.5          # partition 127 kt=2 holds the k=384 dup
    p_ = np.arange(128)[:, None, None]
    kt_ = np.arange(NKT)[None, :, None]
    c_ = np.arange(CPD)[None, None, :]
    in_maps = []
    for d in range(NDEV):
        h16 = np.empty((128, NR16 * W288), np.float16)
        h32 = np.empty((128, 5 * W288), np.float32)
        for ri, r in enumerate(_H16_SRC):
            for kt in range(NKT):
                lo = d * CPD + kt * 128 + 1
                sw = sliding_window_view(Fe16[r, lo:lo + 127 + CPD], CPD)
                h16[:, ri * W288 + kt * CPD:ri * W288 + (kt + 1) * CPD] = sw
        for r in range(5):
            for kt in range(NKT):
                lo = d * CPD + kt * 128 + 1
                sw = sliding_window_view(Fe32[r, lo:lo + 127 + CPD], CPD)
                h32[:, r * W288 + kt * CPD:r * W288 + (kt + 1) * CPD] = sw
        p16 = np.tile(F16[:, d * CPD:(d + 1) * CPD], (1, NKT)).astype(np.float16)
        p32 = np.tile(F32[:, d * CPD:(d + 1) * CPD], (1, NKT))
        i_ = d * CPD + c_
        j_ = (i_ + kt_ * 128 + p_ + 1) % M
        dd = np.empty((128, 2 * W288), np.float16)
        dd[:, 0:W288] = (cx[i_] - cx[j_]).reshape(128, W288)
        dd[:, W288:] = (cy[i_] - cy[j_]).reshape(128, W288)
        in_maps.append({
            'hank16': h16, 'peri16': np.ascontiguousarray(p16),
            'hank32': h32, 'peri32': np.ascontiguousarray(p32),
            'wcolt': wcol, 'ddxy': dd,
        })
    return in_maps


def _combine(partials):
    m = float(M)
    S_iou = sum(float(p[0, 0]) + float(p[1, 0]) for p in partials)
    S_rep = sum(float(p[2, 0]) + float(p[3, 0]) for p in partials)
    S_size = sum(float(p[4, 0]) + float(p[5, 0]) for p in partials)
    return np.array((2.0 * S_rep) / (m * (m - 1.0)) + S_size / m
                    + (2.0 * S_iou) / (m * m), dtype=np.float32)


def kernel(pred):
    from concourse import bass_utils
    if 'nc' not in _PROGRAM_CACHE:
        _PROGRAM_CACHE['nc'] = _build_program()
    nc = _PROGRAM_CACHE['nc']
    in_maps = _prep_inputs(pred)
    res = bass_utils.run_bass_kernel_spmd(nc, in_maps, core_ids=list(range(NDEV)))
    return _combine([r['out'] for r in res.results])


if __name__ == '__main__':
    pred = np.load('/root/problem/pred.npy')
    print('kernel total:', kernel(pred))
